# revision 84
# baseline (speedup 1.0000x reference)
"""Multi-head self-attention TRN2 Bass kernel (8-core SPMD), v2.

Problem: x[2,2048,1024] -> qkv proj (w_qkv[1024,3072]) -> 16-head attention
-> out proj (w_out[1024,1024] + b_out) -> [2,2048,1024], all fp32.

Sharding: core i handles batch b=i//4 and head-group g=i%4 (4 heads each).
Each core computes a partial out-projection (its 256 rows of w_out); the
host sums the 4 partials per batch (fp16 partials, fp32 accumulate) and
adds the bias.

v2 design (vs v1): single fused pipeline per (q-slice, head-pair) unit:
  for kc in 16: S-pair (row-tiled concurrent MMs, both heads -> one
  [128,1024] PSUM tile); one wide exp per kc covering both heads (ACT),
  with a subset of kc offloaded to DVE via a Schraudolph fp16 exp
  (tensor_scalar -> int16 bitcast fp16, ~1.6% elem rms err); O-pair lags
  exp by LAG kc. qkv/v/proj matmuls are interleaved into unit loops to
  keep the PE dense from ~4us (HAM warm) to the end. Out partials are
  written fp16. Normalization uses reciprocal_approx_fast per head.
"""

import sys

if "/opt/trn_rl_repo" not in sys.path:
    sys.path.insert(0, "/opt/trn_rl_repo")

import numpy as np

import concourse.bacc as bacc
import concourse.mybir as mybir
import concourse.tile as tile
from concourse.bass_utils import run_bass_kernel_spmd

F32 = mybir.dt.float32
F16 = mybir.dt.float16
I16 = mybir.dt.int16

N_TOK = 2048
C = 1024
D = 64
CC = C // 128            # 8 contraction chunks
TC = N_TOK // 128        # 16 token chunks
QS = N_TOK // 512        # 4 q-slices
KC = N_TOK // 128        # 16 key chunks
LAG = 2                  # O-pair lags exp by this many kc
DVE_KC = (3, 7, 11, 15)  # kc whose exp runs on DVE (Schraudolph fp16)

# Schraudolph fp16 exp of 0.125*s: bits = rint(A16*s + B16); bitcast int16->fp16
A16 = 0.125 * 1024 * 1.4426950408889634
B16 = 15.0 * 1024 - 39.5

MM_DT = F16

_COMPILED = None


def build_nc(mm_dt=None):
    mm_dt = MM_DT if mm_dt is None else mm_dt
    nc = bacc.Bacc("TRN2", target_bir_lowering=False)

    # xt pre-shuffled on host to [128, QS, CC, 512] so each token-slice DMA
    # reads 8KB-contiguous per-partition lines.
    xt_d = nc.declare_dram_parameter("xt", [128, QS * CC * 512], mm_dt, isOutput=False)
    wq_d = nc.declare_dram_parameter("wq", [128, CC * 256], mm_dt, isOutput=False)
    wk_d = nc.declare_dram_parameter("wk", [128, CC * 256], mm_dt, isOutput=False)
    wv_d = nc.declare_dram_parameter("wv", [128, CC * 256], mm_dt, isOutput=False)
    wo_d = nc.declare_dram_parameter("wo", [128, 2 * C], mm_dt, isOutput=False)
    out_d = nc.declare_dram_parameter("out", [N_TOK, C], F16, isOutput=True)

    with tile.TileContext(nc) as tc:
        with (
            tc.tile_pool(name="const", bufs=1) as const_pool,
            tc.tile_pool(name="xTp", bufs=1) as xt_pool,
            tc.tile_pool(name="w3", bufs=3) as w3_pool,
            tc.tile_pool(name="wop", bufs=1) as wo_pool,
            tc.tile_pool(name="kqt", bufs=1) as kq_pool,
            tc.tile_pool(name="vsb", bufs=1) as v_pool,
            tc.tile_pool(name="otsb", bufs=1) as ot_pool,
            tc.tile_pool(name="pt", bufs=7) as pt_pool,
            tc.tile_pool(name="rcp", bufs=2) as rcp_pool,
            tc.tile_pool(name="outsb", bufs=2) as outp_pool,
            tc.tile_pool(name="stp", bufs=2, space="PSUM") as st_pool,
            tc.tile_pool(name="otp", bufs=2, space="PSUM") as ots_pool,
            tc.tile_pool(name="accp", bufs=2, space="PSUM") as acc_pool,
        ):
            # --- input DMAs, smallest/most-urgent first -------------------
            # wq/wk are p-half-major ([128, 2, CC, 128]) so the first unit's
            # head-pair needs only a 256KB DMA before matmuls can start.
            wq_sb = w3_pool.tile([128, 2, CC, 128], mm_dt, tag="w3")
            wk_sb = w3_pool.tile([128, 2, CC, 128], mm_dt, tag="w3")
            wv_sb = w3_pool.tile([128, CC, 256], mm_dt, tag="w3")
            wo_sb = wo_pool.tile([128, 2, C], mm_dt, tag="wo")
            wk_view = wk_d[:].rearrange("p (g n) -> p g n", g=2)
            wq_view = wq_d[:].rearrange("p (g n) -> p g n", g=2)
            nc.sync.dma_start(
                out=wk_sb[:, 0].rearrange("p c n -> p (c n)"), in_=wk_view[:, 0]
            )
            nc.sync.dma_start(
                out=wq_sb[:, 0].rearrange("p c n -> p (c n)"), in_=wq_view[:, 0]
            )

            xT = xt_pool.tile([128, CC, N_TOK], mm_dt, tag="xT")
            xt_view = xt_d[:].rearrange("p (t c n) -> p t c n", t=QS, c=CC)
            # token-slice-major so the first k/q matmuls can start early
            nc.sync.dma_start(out=xT[:, :, 0:512], in_=xt_view[:, 0])
            nc.sync.dma_start(
                out=wk_sb[:, 1].rearrange("p c n -> p (c n)"), in_=wk_view[:, 1]
            )
            nc.sync.dma_start(
                out=wq_sb[:, 1].rearrange("p c n -> p (c n)"), in_=wq_view[:, 1]
            )
            nc.sync.dma_start(out=wv_sb[:].rearrange("p c n -> p (c n)"), in_=wv_d[:])
            for ts in range(1, QS):
                sl = slice(ts * 512, (ts + 1) * 512)
                nc.sync.dma_start(out=xT[:, :, sl], in_=xt_view[:, ts])
            nc.sync.dma_start(out=wo_sb[:].rearrange("p c n -> p (c n)"), in_=wo_d[:])

            # --- constants / ACT table warm ------------------------------
            ones512 = const_pool.tile([128, 512], mm_dt, tag="ones512")
            nc.vector.memset(ones512[:], 1.0)
            ones256 = ones512[:, 0:256]
            actwarm = const_pool.tile([1, 1], F32, tag="actwarm")
            nc.scalar.activation(
                actwarm[:], ones512[0:1, 0:1], mybir.ActivationFunctionType.Exp
            )
            # dummy matmuls to keep the PE busy through the ~16us input-DMA
            # wall: HAM un-throttles after ~3.4us of sustained activity, so
            # the first real matmuls start at 2.4GHz instead of 1.2.
            for _ in range(4):
                warm_acc = acc_pool.tile([128, 512], F32, tag="acc", name="warm")
                for j in range(8):
                    nc.tensor.matmul(
                        warm_acc[:],
                        ones512[:, 0:128],
                        ones512[:],
                        start=(j == 0),
                        stop=(j == 7),
                    )

            qT = [kq_pool.tile([128, N_TOK], mm_dt, tag=f"qT{p}", name=f"qT{p}") for p in range(2)]
            kT = [kq_pool.tile([128, N_TOK], mm_dt, tag=f"kT{p}", name=f"kT{p}") for p in range(2)]
            oT = [ot_pool.tile([128, N_TOK], mm_dt, tag=f"oT{p}", name=f"oT{p}") for p in range(2)]
            # v_aug: per key-chunk, per head: [64 v columns | 64 ones columns]
            # -> the O^T matmul (M=128) emits softmax denominators on
            # partitions 64..127 of the ots accumulator.
            v_sb = v_pool.tile([128, KC, 4 * 128], mm_dt, tag="v")
            v_aug_view = v_sb[:].rearrange("p k (h e) -> p k h e", h=4)
            for kc in range(KC):
                nc.gpsimd.tensor_copy(
                    v_aug_view[:, kc, :, 64:128],
                    ones256.rearrange("p (h e) -> p h e", h=4),
                )

            # --- emission helpers ---------------------------------------
            def emit_k_slice(p, ts):
                sl = slice(ts * 512, (ts + 1) * 512)
                acc = acc_pool.tile([128, 512], F32, tag="acc", name="acc")
                for cc in range(CC):
                    nc.tensor.matmul(
                        acc[:],
                        wk_sb[:, p, cc, :],
                        xT[:, cc, sl],
                        start=(cc == 0),
                        stop=(cc == CC - 1),
                    )
                nc.vector.tensor_copy(kT[p][:, sl], acc[:])

            def emit_q_slice(p, ts):
                sl = slice(ts * 512, (ts + 1) * 512)
                acc = acc_pool.tile([128, 512], F32, tag="acc", name="acc")
                for cc in range(CC):
                    nc.tensor.matmul(
                        acc[:],
                        wq_sb[:, p, cc, :],
                        xT[:, cc, sl],
                        start=(cc == 0),
                        stop=(cc == CC - 1),
                    )
                nc.vector.tensor_copy(qT[p][:, sl], acc[:])

            def emit_v_kc(kc):
                acc = acc_pool.tile([128, 512], F32, tag="acc", name="acc")
                for cc in range(CC):
                    nc.tensor.matmul(
                        acc[:, 0:256],
                        xT[:, cc, kc * 128:(kc + 1) * 128],
                        wv_sb[:, cc, :],
                        start=(cc == 0),
                        stop=(cc == CC - 1),
                    )
                nc.vector.tensor_copy(
                    v_aug_view[:, kc, :, 0:64],
                    acc[:, 0:256].rearrange("p (h e) -> p h e", h=4),
                )

            def emit_proj_t(t, evict_on_scalar=False, outp=None, dma=True):
                if outp is None:
                    outp = outp_pool.tile([128, C], F16, tag="outp")
                for ns in range(2):
                    po = acc_pool.tile([128, 512], F32, tag="acc", name="po")
                    for p in range(2):
                        nc.tensor.matmul(
                            po[:],
                            oT[p][:, t * 128:(t + 1) * 128],
                            wo_sb[:, p, ns * 512:(ns + 1) * 512],
                            start=(p == 0),
                            stop=(p == 1),
                        )
                    dst = outp[:, ns * 512:(ns + 1) * 512]
                    if evict_on_scalar:
                        nc.scalar.copy(dst, po[:])
                    else:
                        nc.vector.tensor_copy(dst, po[:])
                if dma:
                    nc.sync.dma_start(out=out_d[t * 128:(t + 1) * 128, :], in_=outp[:])

            def emit_proj_pair_fused(t0, alt_evict=True):
                # two token-chunks into one wide tile + a single DMA (the
                # Sync-engine issue cost ~0.6us/DMA dominates small flushes)
                outp2 = outp_pool.tile([128, 2, C], F16, tag="outp2")
                for i, t in enumerate((t0, t0 + 1)):
                    emit_proj_t(t, evict_on_scalar=(alt_evict and i == 1),
                                outp=outp2[:, i], dma=False)
                nc.sync.dma_start(
                    out=out_d[:].rearrange("(c p) n -> p c n", p=128)[
                        :, t0:t0 + 2, :
                    ],
                    in_=outp2[:],
                )

            def unit(ts, p, extras=None, dve_kc=DVE_KC, norm_hooks=None,
                     tail_warm=0):
                """One (q-slice, head-pair) attention unit, fused pipeline.

                norm_hooks: optional pair of callables; normalization is then
                emitted in two 256-token halves with hooks[i] run after half
                i (lets tail proj chunks start before the full normalize).
                """
                extras = extras or {}
                sl = slice(ts * 512, (ts + 1) * 512)
                ots = [
                    ots_pool.tile([128, 512], F32, tag="ots", name=f"ot{h}")
                    for h in range(2)
                ]
                pend = {}

                def emit_o_pair(kc):
                    pt = pend.pop(kc)
                    for h in range(2):
                        a = 2 * p + h
                        nc.tensor.matmul(
                            ots[h][:],
                            v_sb[:, kc, a * 128:(a + 1) * 128],
                            pt[:, h * 512:(h + 1) * 512].bitcast(mm_dt),
                            start=(kc == 0),
                            stop=(kc == KC - 1),
                        )

                # blocked in pairs of kc: S-pairs (64-row mode) grouped, then
                # O-pairs + interleaves (128-row mode) grouped — each
                # 64<->128-row mode switch costs ~100ns of unhidden
                # LDWEIGHTS. Per-kc st tiles keep exp/slot pipelining fine-
                # grained so the PE is not serialized to ACT block pace.
                for b in range(KC // 2):
                    if b >= 2:
                        emit_o_pair(2 * b - 4)
                        emit_o_pair(2 * b - 3)
                    for kc in (2 * b, 2 * b + 1):
                        st = st_pool.tile([128, 1024], F32, tag="st", name="st")
                        for h in range(2):
                            hp = slice(h * 64, (h + 1) * 64)
                            nc.tensor.matmul(
                                st[:, h * 512:(h + 1) * 512],
                                kT[p][hp, kc * 128:(kc + 1) * 128],
                                qT[p][hp, sl],
                                start=True,
                                stop=True,
                            )
                        pt = pt_pool.tile([128, 1024], F16, tag="pt", name="pt")
                        if kc in dve_kc:
                            nc.vector.tensor_scalar(
                                pt[:].bitcast(I16),
                                st[:],
                                A16,
                                B16,
                                mybir.AluOpType.mult,
                                mybir.AluOpType.add,
                            )
                        else:
                            nc.scalar.activation(
                                pt[:],
                                st[:],
                                mybir.ActivationFunctionType.Exp,
                                scale=0.125,
                            )
                        pend[kc] = pt
                    for f in extras.get(b, ()):
                        f()
                for kc in range(KC - 4, KC):
                    emit_o_pair(kc)
                # optional dummy matmuls so the PE stays HAM-warm while the
                # normalize chain runs on DVE (else the tail projs run cold)
                if tail_warm:
                    twarm = acc_pool.tile([128, 512], F32, tag="acc", name="tw")
                    for j in range(tail_warm):
                        nc.tensor.matmul(
                            twarm[:],
                            ones512[:, 0:128],
                            ones512[:],
                            start=(j == 0),
                            stop=(j == tail_warm - 1),
                        )

                # normalize (proven ops only): pack numer/sums across heads
                # with partition-crossing copies, then one reciprocal + mul.
                halves = [(0, 512)] if norm_hooks is None else [(0, 256), (256, 512)]
                for i, (c0, c1) in enumerate(halves):
                    numer = rcp_pool.tile([128, 512], F32, tag="numer")
                    sums = rcp_pool.tile([128, 512], F32, tag="sums")
                    for h in range(2):
                        hq = slice(h * 64, (h + 1) * 64)
                        nc.vector.tensor_copy(numer[hq, c0:c1], ots[h][0:64, c0:c1])
                        nc.vector.tensor_copy(sums[hq, c0:c1], ots[h][64:128, c0:c1])
                    rcp = rcp_pool.tile([128, 512], F32, tag="rcp")
                    nc.vector.reciprocal_approx_fast(
                        out=rcp[:, c0:c1], in_=sums[:, c0:c1]
                    )
                    nc.vector.tensor_mul(
                        oT[p][:, ts * 512 + c0:ts * 512 + c1],
                        numer[:, c0:c1],
                        rcp[:, c0:c1],
                    )
                    if norm_hooks is not None:
                        norm_hooks[i]()

            # --- master schedule ----------------------------------------
            emit_k_slice(0, 0)
            emit_q_slice(0, 0)

            # extras are keyed by block index b (= kc pair 2b, 2b+1)
            # A = (ts0, p0): absorbs all v chunks + rest of k(p0)/q(p0,1)
            unit(0, 0, {
                0: [lambda: emit_v_kc(0), lambda: emit_k_slice(0, 1),
                    lambda: emit_v_kc(1)],
                1: [lambda: emit_v_kc(2), lambda: emit_k_slice(0, 2),
                    lambda: emit_v_kc(3)],
                2: [lambda: emit_v_kc(4), lambda: emit_k_slice(0, 3),
                    lambda: emit_v_kc(5)],
                3: [lambda: emit_q_slice(0, 1), lambda: emit_v_kc(6),
                    lambda: emit_v_kc(7)],
                4: [lambda: emit_v_kc(8), lambda: emit_v_kc(9)],
                5: [lambda: emit_v_kc(10), lambda: emit_v_kc(11)],
                6: [lambda: emit_v_kc(12), lambda: emit_v_kc(13)],
                7: [lambda: emit_v_kc(14), lambda: emit_v_kc(15)],
            }, dve_kc=())
            # B = (ts1, p0): absorbs k(p1) + q(p1,0) + q(p0,2)
            unit(1, 0, {
                0: [lambda: emit_k_slice(1, 0)],
                1: [lambda: emit_k_slice(1, 1)],
                2: [lambda: emit_k_slice(1, 2)],
                3: [lambda: emit_k_slice(1, 3)],
                4: [lambda: emit_q_slice(1, 0)],
                5: [lambda: emit_q_slice(0, 2)],
            }, dve_kc=(5, 11))
            # C = (ts0, p1)
            unit(0, 1, {
                1: [lambda: emit_q_slice(0, 3)],
                4: [lambda: emit_q_slice(1, 1)],
            }, dve_kc=(3, 6, 9, 12, 14))
            # D = (ts2, p0): proj(0) interleaved
            unit(2, 0, {
                0: [lambda: emit_proj_t(0)],
                2: [lambda: emit_proj_t(1)],
                4: [lambda: emit_proj_t(2)],
                5: [lambda: emit_proj_t(3)],
                6: [lambda: emit_q_slice(1, 2)],
            }, dve_kc=(3, 6, 9, 12, 14))
            # E = (ts1, p1)
            unit(1, 1, {
                1: [lambda: emit_q_slice(1, 3)],
            }, dve_kc=(3, 6, 9, 12, 14))
            # F = (ts3, p0): proj(1) interleaved
            unit(3, 0, {
                0: [lambda: emit_proj_t(4)],
                2: [lambda: emit_proj_t(5)],
                4: [lambda: emit_proj_t(6)],
                5: [lambda: emit_proj_t(7)],
            })
            # G = (ts2, p1): mid-unit DVE offload (DVE FIFO is free mid-unit)
            unit(2, 1, {}, dve_kc=(1, 3, 5, 7, 9, 12, 14))
            # H = (ts3, p1): proj(2) interleaved, evicts split across ACT/DVE
            unit(3, 1, {
                0: [lambda: emit_proj_t(8)],
                2: [lambda: emit_proj_t(9, evict_on_scalar=True)],
                4: [lambda: emit_proj_t(10)],
                5: [lambda: emit_proj_t(11, evict_on_scalar=True)],
            }, dve_kc=(1, 3, 5, 7, 9, 11, 13, 15), norm_hooks=(
                # tail proj(3): two fused token-chunks per normalize half,
                # evicts alternating DVE/ACT, one DMA per pair.
                lambda: emit_proj_pair_fused(12),
                lambda: emit_proj_pair_fused(14),
            ), tail_warm=14)

    nc.compile()
    return nc


def _shard_inputs(x, w_qkv, w_out):
    # [2048, 1024] -> [128, QS*CC*512]: partition p holds, per token-slice,
    # per contraction chunk, 512 contiguous tokens (8KB lines per DMA).
    xts = [
        np.ascontiguousarray(
            x[b].reshape(QS, 512, CC, 128).transpose(3, 0, 2, 1).reshape(128, -1)
        ).astype(np.float16)
        for b in range(2)
    ]
    in_maps = []
    for i in range(8):
        b, g = divmod(i, 4)
        cs = slice(256 * g, 256 * (g + 1))

        def shuf(w):
            # [1024, n] -> [128, CC * n] with chunk-major columns
            n = w.shape[1]
            return np.ascontiguousarray(
                w.reshape(CC, 128, n).transpose(1, 0, 2).reshape(128, CC * n)
            ).astype(np.float16)

        def shuf_ph(w):
            # [1024, 256] -> [128, 2*CC*128], p-half-major then chunk-major
            return np.ascontiguousarray(
                w.reshape(CC, 128, 2, 128).transpose(1, 2, 0, 3).reshape(128, -1)
            ).astype(np.float16)

        in_maps.append({
            "xt": xts[b],
            "wq": shuf_ph(w_qkv[:, cs]),
            "wk": shuf_ph(w_qkv[:, 1024 + 256 * g:1024 + 256 * (g + 1)]),
            "wv": shuf(w_qkv[:, 2048 + 256 * g:2048 + 256 * (g + 1)]),
            "wo": np.ascontiguousarray(
                w_out[cs, :].reshape(2, 128, 1024).transpose(1, 0, 2).reshape(128, 2048)
            ).astype(np.float16),
        })
    return in_maps


def kernel(x, w_qkv, w_out, b_out):
    global _COMPILED
    x = np.asarray(x, np.float32)
    w_qkv = np.asarray(w_qkv, np.float32)
    w_out = np.asarray(w_out, np.float32)
    b_out = np.asarray(b_out, np.float32)

    if _COMPILED is None:
        _COMPILED = build_nc()
    nc = _COMPILED

    in_maps = _shard_inputs(x, w_qkv, w_out)
    res = run_bass_kernel_spmd(nc, in_maps, core_ids=list(range(8)))
    out = np.zeros((2, N_TOK, C), np.float32)
    for i in range(8):
        b = i // 4
        out[b] += res.results[i]["out"].astype(np.float32)
    out += b_out[None, None, :]
    return out


# revision 85
# speedup vs baseline: 1.0050x; 1.0050x over previous
"""Multi-head self-attention TRN2 Bass kernel (8-core SPMD), v2.

Problem: x[2,2048,1024] -> qkv proj (w_qkv[1024,3072]) -> 16-head attention
-> out proj (w_out[1024,1024] + b_out) -> [2,2048,1024], all fp32.

Sharding: core i handles batch b=i//4 and head-group g=i%4 (4 heads each).
Each core computes a partial out-projection (its 256 rows of w_out); the
host sums the 4 partials per batch (fp16 partials, fp32 accumulate) and
adds the bias.

v2 design (vs v1): single fused pipeline per (q-slice, head-pair) unit:
  for kc in 16: S-pair (row-tiled concurrent MMs, both heads -> one
  [128,1024] PSUM tile); one wide exp per kc covering both heads (ACT),
  with a subset of kc offloaded to DVE via a Schraudolph fp16 exp
  (tensor_scalar -> int16 bitcast fp16, ~1.6% elem rms err); O-pair lags
  exp by LAG kc. qkv/v/proj matmuls are interleaved into unit loops to
  keep the PE dense from ~4us (HAM warm) to the end. Out partials are
  written fp16. Normalization uses reciprocal_approx_fast per head.
"""

import sys

if "/opt/trn_rl_repo" not in sys.path:
    sys.path.insert(0, "/opt/trn_rl_repo")

import numpy as np

import concourse.bacc as bacc
import concourse.mybir as mybir
import concourse.tile as tile
from concourse.bass_utils import run_bass_kernel_spmd

F32 = mybir.dt.float32
F16 = mybir.dt.float16
I16 = mybir.dt.int16

N_TOK = 2048
C = 1024
D = 64
CC = C // 128            # 8 contraction chunks
TC = N_TOK // 128        # 16 token chunks
QS = N_TOK // 512        # 4 q-slices
KC = N_TOK // 128        # 16 key chunks
LAG = 2                  # O-pair lags exp by this many kc
DVE_KC = (3, 7, 11, 15)  # kc whose exp runs on DVE (Schraudolph fp16)

# Schraudolph fp16 exp of 0.125*s: bits = rint(A16*s + B16); bitcast int16->fp16
A16 = 0.125 * 1024 * 1.4426950408889634
B16 = 15.0 * 1024 - 39.5

MM_DT = F16

_COMPILED = None


def build_nc(mm_dt=None):
    mm_dt = MM_DT if mm_dt is None else mm_dt
    nc = bacc.Bacc("TRN2", target_bir_lowering=False)

    # xt pre-shuffled on host to [128, QS, CC, 512] so each token-slice DMA
    # reads 8KB-contiguous per-partition lines.
    xt_d = nc.declare_dram_parameter("xt", [128, QS * CC * 512], mm_dt, isOutput=False)
    wq_d = nc.declare_dram_parameter("wq", [128, CC * 256], mm_dt, isOutput=False)
    wk_d = nc.declare_dram_parameter("wk", [128, CC * 256], mm_dt, isOutput=False)
    wv_d = nc.declare_dram_parameter("wv", [128, CC * 256], mm_dt, isOutput=False)
    wo_d = nc.declare_dram_parameter("wo", [128, 2 * C], mm_dt, isOutput=False)
    out_d = nc.declare_dram_parameter("out", [N_TOK, C], F16, isOutput=True)

    with tile.TileContext(nc) as tc:
        with (
            tc.tile_pool(name="const", bufs=1) as const_pool,
            tc.tile_pool(name="xTp", bufs=1) as xt_pool,
            tc.tile_pool(name="w3", bufs=3) as w3_pool,
            tc.tile_pool(name="wop", bufs=1) as wo_pool,
            tc.tile_pool(name="kqt", bufs=1) as kq_pool,
            tc.tile_pool(name="vsb", bufs=1) as v_pool,
            tc.tile_pool(name="otsb", bufs=1) as ot_pool,
            tc.tile_pool(name="pt", bufs=7) as pt_pool,
            tc.tile_pool(name="rcp", bufs=2) as rcp_pool,
            tc.tile_pool(name="outsb", bufs=2) as outp_pool,
            tc.tile_pool(name="stp", bufs=2, space="PSUM") as st_pool,
            tc.tile_pool(name="otp", bufs=2, space="PSUM") as ots_pool,
            tc.tile_pool(name="accp", bufs=2, space="PSUM") as acc_pool,
        ):
            # --- input DMAs, smallest/most-urgent first -------------------
            # wq/wk are p-half-major ([128, 2, CC, 128]) so the first unit's
            # head-pair needs only a 256KB DMA before matmuls can start.
            wq_sb = w3_pool.tile([128, 2, CC, 128], mm_dt, tag="w3")
            wk_sb = w3_pool.tile([128, 2, CC, 128], mm_dt, tag="w3")
            wv_sb = w3_pool.tile([128, CC, 256], mm_dt, tag="w3")
            wo_sb = wo_pool.tile([128, 2, C], mm_dt, tag="wo")
            wk_view = wk_d[:].rearrange("p (g n) -> p g n", g=2)
            wq_view = wq_d[:].rearrange("p (g n) -> p g n", g=2)
            nc.sync.dma_start(
                out=wk_sb[:, 0].rearrange("p c n -> p (c n)"), in_=wk_view[:, 0]
            )
            nc.sync.dma_start(
                out=wq_sb[:, 0].rearrange("p c n -> p (c n)"), in_=wq_view[:, 0]
            )

            xT = xt_pool.tile([128, CC, N_TOK], mm_dt, tag="xT")
            xt_view = xt_d[:].rearrange("p (t c n) -> p t c n", t=QS, c=CC)
            # token-slice-major so the first k/q matmuls can start early
            nc.sync.dma_start(out=xT[:, :, 0:512], in_=xt_view[:, 0])
            nc.sync.dma_start(
                out=wk_sb[:, 1].rearrange("p c n -> p (c n)"), in_=wk_view[:, 1]
            )
            nc.sync.dma_start(
                out=wq_sb[:, 1].rearrange("p c n -> p (c n)"), in_=wq_view[:, 1]
            )
            nc.sync.dma_start(out=wv_sb[:].rearrange("p c n -> p (c n)"), in_=wv_d[:])
            for ts in range(1, QS):
                sl = slice(ts * 512, (ts + 1) * 512)
                nc.sync.dma_start(out=xT[:, :, sl], in_=xt_view[:, ts])
            nc.sync.dma_start(out=wo_sb[:].rearrange("p c n -> p (c n)"), in_=wo_d[:])

            # --- constants / ACT table warm ------------------------------
            ones512 = const_pool.tile([128, 512], mm_dt, tag="ones512")
            nc.vector.memset(ones512[:], 1.0)
            ones256 = ones512[:, 0:256]
            actwarm = const_pool.tile([1, 1], F32, tag="actwarm")
            nc.scalar.activation(
                actwarm[:], ones512[0:1, 0:1], mybir.ActivationFunctionType.Exp
            )
            # dummy matmuls to keep the PE busy through the ~16us input-DMA
            # wall: HAM un-throttles after ~3.4us of sustained activity, so
            # the first real matmuls start at 2.4GHz instead of 1.2.
            for _ in range(4):
                warm_acc = acc_pool.tile([128, 512], F32, tag="acc", name="warm")
                for j in range(8):
                    nc.tensor.matmul(
                        warm_acc[:],
                        ones512[:, 0:128],
                        ones512[:],
                        start=(j == 0),
                        stop=(j == 7),
                    )

            qT = [kq_pool.tile([128, N_TOK], mm_dt, tag=f"qT{p}", name=f"qT{p}") for p in range(2)]
            kT = [kq_pool.tile([128, N_TOK], mm_dt, tag=f"kT{p}", name=f"kT{p}") for p in range(2)]
            oT = [ot_pool.tile([128, N_TOK], mm_dt, tag=f"oT{p}", name=f"oT{p}") for p in range(2)]
            # v_aug: per key-chunk, per head: [64 v columns | 64 ones columns]
            # -> the O^T matmul (M=128) emits softmax denominators on
            # partitions 64..127 of the ots accumulator.
            v_sb = v_pool.tile([128, KC, 4 * 128], mm_dt, tag="v")
            v_aug_view = v_sb[:].rearrange("p k (h e) -> p k h e", h=4)
            for kc in range(KC):
                nc.gpsimd.tensor_copy(
                    v_aug_view[:, kc, :, 64:128],
                    ones256.rearrange("p (h e) -> p h e", h=4),
                )

            # --- emission helpers ---------------------------------------
            def emit_k_slice(p, ts):
                sl = slice(ts * 512, (ts + 1) * 512)
                acc = acc_pool.tile([128, 512], F32, tag="acc", name="acc")
                for cc in range(CC):
                    nc.tensor.matmul(
                        acc[:],
                        wk_sb[:, p, cc, :],
                        xT[:, cc, sl],
                        start=(cc == 0),
                        stop=(cc == CC - 1),
                    )
                nc.vector.tensor_copy(kT[p][:, sl], acc[:])

            def emit_q_slice(p, ts):
                sl = slice(ts * 512, (ts + 1) * 512)
                acc = acc_pool.tile([128, 512], F32, tag="acc", name="acc")
                for cc in range(CC):
                    nc.tensor.matmul(
                        acc[:],
                        wq_sb[:, p, cc, :],
                        xT[:, cc, sl],
                        start=(cc == 0),
                        stop=(cc == CC - 1),
                    )
                nc.vector.tensor_copy(qT[p][:, sl], acc[:])

            def emit_v_kc(kc):
                acc = acc_pool.tile([128, 512], F32, tag="acc", name="acc")
                for cc in range(CC):
                    nc.tensor.matmul(
                        acc[:, 0:256],
                        xT[:, cc, kc * 128:(kc + 1) * 128],
                        wv_sb[:, cc, :],
                        start=(cc == 0),
                        stop=(cc == CC - 1),
                    )
                nc.vector.tensor_copy(
                    v_aug_view[:, kc, :, 0:64],
                    acc[:, 0:256].rearrange("p (h e) -> p h e", h=4),
                )

            def emit_proj_t(t, evict_on_scalar=False, outp=None, dma=True):
                if outp is None:
                    outp = outp_pool.tile([128, C], F16, tag="outp")
                for ns in range(2):
                    po = acc_pool.tile([128, 512], F32, tag="acc", name="po")
                    for p in range(2):
                        nc.tensor.matmul(
                            po[:],
                            oT[p][:, t * 128:(t + 1) * 128],
                            wo_sb[:, p, ns * 512:(ns + 1) * 512],
                            start=(p == 0),
                            stop=(p == 1),
                        )
                    dst = outp[:, ns * 512:(ns + 1) * 512]
                    if evict_on_scalar:
                        nc.scalar.copy(dst, po[:])
                    else:
                        nc.vector.tensor_copy(dst, po[:])
                if dma:
                    nc.sync.dma_start(out=out_d[t * 128:(t + 1) * 128, :], in_=outp[:])

            def emit_proj_pair_fused(t0, alt_evict=True):
                # two token-chunks into one wide tile + a single DMA (the
                # Sync-engine issue cost ~0.6us/DMA dominates small flushes)
                outp2 = outp_pool.tile([128, 2, C], F16, tag="outp2")
                for i, t in enumerate((t0, t0 + 1)):
                    emit_proj_t(t, evict_on_scalar=(alt_evict and i == 1),
                                outp=outp2[:, i], dma=False)
                nc.sync.dma_start(
                    out=out_d[:].rearrange("(c p) n -> p c n", p=128)[
                        :, t0:t0 + 2, :
                    ],
                    in_=outp2[:],
                )

            def unit(ts, p, extras=None, dve_kc=DVE_KC, norm_hooks=None,
                     tail_warm=0):
                """One (q-slice, head-pair) attention unit, fused pipeline.

                norm_hooks: optional pair of callables; normalization is then
                emitted in two 256-token halves with hooks[i] run after half
                i (lets tail proj chunks start before the full normalize).
                """
                extras = extras or {}
                sl = slice(ts * 512, (ts + 1) * 512)
                ots = [
                    ots_pool.tile([128, 512], F32, tag="ots", name=f"ot{h}")
                    for h in range(2)
                ]
                pend = {}

                def emit_o_pair(kc):
                    pt = pend.pop(kc)
                    for h in range(2):
                        a = 2 * p + h
                        nc.tensor.matmul(
                            ots[h][:],
                            v_sb[:, kc, a * 128:(a + 1) * 128],
                            pt[:, h * 512:(h + 1) * 512].bitcast(mm_dt),
                            start=(kc == 0),
                            stop=(kc == KC - 1),
                        )

                # blocked in pairs of kc: S-pairs (64-row mode) grouped, then
                # O-pairs + interleaves (128-row mode) grouped — each
                # 64<->128-row mode switch costs ~100ns of unhidden
                # LDWEIGHTS. Per-kc st tiles keep exp/slot pipelining fine-
                # grained so the PE is not serialized to ACT block pace.
                for b in range(KC // 2):
                    if b >= 2:
                        emit_o_pair(2 * b - 4)
                        emit_o_pair(2 * b - 3)
                    for kc in (2 * b, 2 * b + 1):
                        st = st_pool.tile([128, 1024], F32, tag="st", name="st")
                        for h in range(2):
                            hp = slice(h * 64, (h + 1) * 64)
                            nc.tensor.matmul(
                                st[:, h * 512:(h + 1) * 512],
                                kT[p][hp, kc * 128:(kc + 1) * 128],
                                qT[p][hp, sl],
                                start=True,
                                stop=True,
                            )
                        pt = pt_pool.tile([128, 1024], F16, tag="pt", name="pt")
                        if kc in dve_kc:
                            nc.vector.tensor_scalar(
                                pt[:].bitcast(I16),
                                st[:],
                                A16,
                                B16,
                                mybir.AluOpType.mult,
                                mybir.AluOpType.add,
                            )
                        else:
                            nc.scalar.activation(
                                pt[:],
                                st[:],
                                mybir.ActivationFunctionType.Exp,
                                scale=0.125,
                            )
                        pend[kc] = pt
                    for f in extras.get(b, ()):
                        f()
                for kc in range(KC - 4, KC):
                    emit_o_pair(kc)
                # optional dummy matmuls so the PE stays HAM-warm while the
                # normalize chain runs on DVE (else the tail projs run cold)
                if tail_warm:
                    twarm = acc_pool.tile([128, 512], F32, tag="acc", name="tw")
                    for j in range(tail_warm):
                        nc.tensor.matmul(
                            twarm[:],
                            ones512[:, 0:128],
                            ones512[:],
                            start=(j == 0),
                            stop=(j == tail_warm - 1),
                        )

                # normalize (proven ops only): pack numer/sums across heads
                # with partition-crossing copies, then one reciprocal + mul.
                halves = [(0, 512)] if norm_hooks is None else [(0, 256), (256, 512)]
                for i, (c0, c1) in enumerate(halves):
                    numer = rcp_pool.tile([128, 512], F32, tag="numer")
                    sums = rcp_pool.tile([128, 512], F32, tag="sums")
                    for h in range(2):
                        hq = slice(h * 64, (h + 1) * 64)
                        nc.vector.tensor_copy(numer[hq, c0:c1], ots[h][0:64, c0:c1])
                        nc.vector.tensor_copy(sums[hq, c0:c1], ots[h][64:128, c0:c1])
                    rcp = rcp_pool.tile([128, 512], F32, tag="rcp")
                    nc.vector.reciprocal_approx_fast(
                        out=rcp[:, c0:c1], in_=sums[:, c0:c1]
                    )
                    nc.vector.tensor_mul(
                        oT[p][:, ts * 512 + c0:ts * 512 + c1],
                        numer[:, c0:c1],
                        rcp[:, c0:c1],
                    )
                    if norm_hooks is not None:
                        norm_hooks[i]()

            # --- master schedule ----------------------------------------
            emit_k_slice(0, 0)
            emit_q_slice(0, 0)

            # extras are keyed by block index b (= kc pair 2b, 2b+1)
            # A = (ts0, p0): absorbs all v chunks + rest of k(p0)/q(p0,1)
            unit(0, 0, {
                0: [lambda: emit_v_kc(0), lambda: emit_k_slice(0, 1),
                    lambda: emit_v_kc(1)],
                1: [lambda: emit_v_kc(2), lambda: emit_k_slice(0, 2),
                    lambda: emit_v_kc(3)],
                2: [lambda: emit_v_kc(4), lambda: emit_k_slice(0, 3),
                    lambda: emit_v_kc(5)],
                3: [lambda: emit_q_slice(0, 1), lambda: emit_v_kc(6),
                    lambda: emit_v_kc(7)],
                4: [lambda: emit_v_kc(8), lambda: emit_v_kc(9)],
                5: [lambda: emit_v_kc(10), lambda: emit_v_kc(11)],
                6: [lambda: emit_v_kc(12), lambda: emit_v_kc(13)],
                7: [lambda: emit_v_kc(14), lambda: emit_v_kc(15)],
            })
            # B = (ts1, p0): absorbs k(p1) + q(p1,0) + q(p0,2)
            unit(1, 0, {
                0: [lambda: emit_k_slice(1, 0)],
                1: [lambda: emit_k_slice(1, 1)],
                2: [lambda: emit_k_slice(1, 2)],
                3: [lambda: emit_k_slice(1, 3)],
                4: [lambda: emit_q_slice(1, 0)],
                5: [lambda: emit_q_slice(0, 2)],
            })
            # C = (ts0, p1)
            unit(0, 1, {
                1: [lambda: emit_q_slice(0, 3)],
                4: [lambda: emit_q_slice(1, 1)],
            }, dve_kc=(3, 6, 9, 12, 14))
            # D = (ts2, p0): proj(0) interleaved
            unit(2, 0, {
                0: [lambda: emit_proj_t(0)],
                2: [lambda: emit_proj_t(1)],
                4: [lambda: emit_proj_t(2)],
                5: [lambda: emit_proj_t(3)],
                6: [lambda: emit_q_slice(1, 2)],
            }, dve_kc=(3, 6, 9, 12, 14))
            # E = (ts1, p1)
            unit(1, 1, {
                1: [lambda: emit_q_slice(1, 3)],
            }, dve_kc=(3, 6, 9, 12, 14))
            # F = (ts3, p0): proj(1) interleaved
            unit(3, 0, {
                0: [lambda: emit_proj_t(4)],
                2: [lambda: emit_proj_t(5)],
                4: [lambda: emit_proj_t(6)],
                5: [lambda: emit_proj_t(7)],
            })
            # G = (ts2, p1): mid-unit DVE offload (DVE FIFO is free mid-unit)
            unit(2, 1, {}, dve_kc=(1, 3, 5, 7, 9, 12, 14))
            # H = (ts3, p1): proj(2) interleaved, evicts split across ACT/DVE
            unit(3, 1, {
                0: [lambda: emit_proj_t(8)],
                2: [lambda: emit_proj_t(9, evict_on_scalar=True)],
                4: [lambda: emit_proj_t(10)],
                5: [lambda: emit_proj_t(11, evict_on_scalar=True)],
            }, dve_kc=(1, 3, 5, 7, 9, 11, 13, 15), norm_hooks=(
                # tail proj(3): two fused token-chunks per normalize half,
                # evicts alternating DVE/ACT, one DMA per pair.
                lambda: emit_proj_pair_fused(12),
                lambda: emit_proj_pair_fused(14),
            ), tail_warm=14)

    nc.compile()
    return nc


def _shard_inputs(x, w_qkv, w_out):
    # [2048, 1024] -> [128, QS*CC*512]: partition p holds, per token-slice,
    # per contraction chunk, 512 contiguous tokens (8KB lines per DMA).
    xts = [
        np.ascontiguousarray(
            x[b].reshape(QS, 512, CC, 128).transpose(3, 0, 2, 1).reshape(128, -1)
        ).astype(np.float16)
        for b in range(2)
    ]
    in_maps = []
    for i in range(8):
        b, g = divmod(i, 4)
        cs = slice(256 * g, 256 * (g + 1))

        def shuf(w):
            # [1024, n] -> [128, CC * n] with chunk-major columns
            n = w.shape[1]
            return np.ascontiguousarray(
                w.reshape(CC, 128, n).transpose(1, 0, 2).reshape(128, CC * n)
            ).astype(np.float16)

        def shuf_ph(w):
            # [1024, 256] -> [128, 2*CC*128], p-half-major then chunk-major
            return np.ascontiguousarray(
                w.reshape(CC, 128, 2, 128).transpose(1, 2, 0, 3).reshape(128, -1)
            ).astype(np.float16)

        in_maps.append({
            "xt": xts[b],
            "wq": shuf_ph(w_qkv[:, cs]),
            "wk": shuf_ph(w_qkv[:, 1024 + 256 * g:1024 + 256 * (g + 1)]),
            "wv": shuf(w_qkv[:, 2048 + 256 * g:2048 + 256 * (g + 1)]),
            "wo": np.ascontiguousarray(
                w_out[cs, :].reshape(2, 128, 1024).transpose(1, 0, 2).reshape(128, 2048)
            ).astype(np.float16),
        })
    return in_maps


def kernel(x, w_qkv, w_out, b_out):
    global _COMPILED
    x = np.asarray(x, np.float32)
    w_qkv = np.asarray(w_qkv, np.float32)
    w_out = np.asarray(w_out, np.float32)
    b_out = np.asarray(b_out, np.float32)

    if _COMPILED is None:
        _COMPILED = build_nc()
    nc = _COMPILED

    in_maps = _shard_inputs(x, w_qkv, w_out)
    res = run_bass_kernel_spmd(nc, in_maps, core_ids=list(range(8)))
    out = np.zeros((2, N_TOK, C), np.float32)
    for i in range(8):
        b = i // 4
        out[b] += res.results[i]["out"].astype(np.float32)
    out += b_out[None, None, :]
    return out


# revision 87
# speedup vs baseline: 1.0206x; 1.0154x over previous
"""Multi-head self-attention TRN2 Bass kernel (8-core SPMD), v2.

Problem: x[2,2048,1024] -> qkv proj (w_qkv[1024,3072]) -> 16-head attention
-> out proj (w_out[1024,1024] + b_out) -> [2,2048,1024], all fp32.

Sharding: core i handles batch b=i//4 and head-group g=i%4 (4 heads each).
Each core computes a partial out-projection (its 256 rows of w_out); the
host sums the 4 partials per batch (fp16 partials, fp32 accumulate) and
adds the bias.

v2 design (vs v1): single fused pipeline per (q-slice, head-pair) unit:
  for kc in 16: S-pair (row-tiled concurrent MMs, both heads -> one
  [128,1024] PSUM tile); one wide exp per kc covering both heads (ACT),
  with a subset of kc offloaded to DVE via a Schraudolph fp16 exp
  (tensor_scalar -> int16 bitcast fp16, ~1.6% elem rms err); O-pair lags
  exp by LAG kc. qkv/v/proj matmuls are interleaved into unit loops to
  keep the PE dense from ~4us (HAM warm) to the end. Out partials are
  written fp16. Normalization uses reciprocal_approx_fast per head.
"""

import sys

if "/opt/trn_rl_repo" not in sys.path:
    sys.path.insert(0, "/opt/trn_rl_repo")

import numpy as np

import concourse.bacc as bacc
import concourse.mybir as mybir
import concourse.tile as tile
from concourse.bass_utils import run_bass_kernel_spmd

F32 = mybir.dt.float32
F16 = mybir.dt.float16
I16 = mybir.dt.int16

N_TOK = 2048
C = 1024
D = 64
CC = C // 128            # 8 contraction chunks
TC = N_TOK // 128        # 16 token chunks
QS = N_TOK // 512        # 4 q-slices
KC = N_TOK // 128        # 16 key chunks
LAG = 2                  # O-pair lags exp by this many kc
DVE_KC = (3, 7, 11, 15)  # kc whose exp runs on DVE (Schraudolph fp16)

# Schraudolph fp16 exp of 0.125*s: bits = rint(A16*s + B16); bitcast int16->fp16
A16 = 0.125 * 1024 * 1.4426950408889634
B16 = 15.0 * 1024 - 39.5

MM_DT = F16

_COMPILED = None


def build_nc(mm_dt=None):
    mm_dt = MM_DT if mm_dt is None else mm_dt
    nc = bacc.Bacc("TRN2", target_bir_lowering=False)

    # xt pre-shuffled on host to [128, QS, CC, 512] so each token-slice DMA
    # reads 8KB-contiguous per-partition lines.
    xt_d = nc.declare_dram_parameter("xt", [128, QS * CC * 512], mm_dt, isOutput=False)
    wq_d = nc.declare_dram_parameter("wq", [128, CC * 256], mm_dt, isOutput=False)
    wk_d = nc.declare_dram_parameter("wk", [128, CC * 256], mm_dt, isOutput=False)
    wv_d = nc.declare_dram_parameter("wv", [128, CC * 256], mm_dt, isOutput=False)
    wo_d = nc.declare_dram_parameter("wo", [128, 2 * C], mm_dt, isOutput=False)
    out_d = nc.declare_dram_parameter("out", [N_TOK, C], F16, isOutput=True)

    with tile.TileContext(nc) as tc:
        with (
            tc.tile_pool(name="const", bufs=1) as const_pool,
            tc.tile_pool(name="xTp", bufs=1) as xt_pool,
            tc.tile_pool(name="w3", bufs=3) as w3_pool,
            tc.tile_pool(name="wop", bufs=1) as wo_pool,
            tc.tile_pool(name="kqt", bufs=1) as kq_pool,
            tc.tile_pool(name="vsb", bufs=1) as v_pool,
            tc.tile_pool(name="otsb", bufs=1) as ot_pool,
            tc.tile_pool(name="pt", bufs=7) as pt_pool,
            tc.tile_pool(name="rcp", bufs=2) as rcp_pool,
            tc.tile_pool(name="outsb", bufs=2) as outp_pool,
            tc.tile_pool(name="stp", bufs=2, space="PSUM") as st_pool,
            tc.tile_pool(name="otp", bufs=2, space="PSUM") as ots_pool,
            tc.tile_pool(name="accp", bufs=2, space="PSUM") as acc_pool,
        ):
            # --- input DMAs, smallest/most-urgent first -------------------
            # wq/wk are p-half-major ([128, 2, CC, 128]) so the first unit's
            # head-pair needs only a 256KB DMA before matmuls can start.
            wq_sb = w3_pool.tile([128, 2, CC, 128], mm_dt, tag="w3")
            wk_sb = w3_pool.tile([128, 2, CC, 128], mm_dt, tag="w3")
            wv_sb = w3_pool.tile([128, CC, 256], mm_dt, tag="w3")
            wo_sb = wo_pool.tile([128, 2, C], mm_dt, tag="wo")
            wk_view = wk_d[:].rearrange("p (g n) -> p g n", g=2)
            wq_view = wq_d[:].rearrange("p (g n) -> p g n", g=2)
            nc.sync.dma_start(
                out=wk_sb[:, 0].rearrange("p c n -> p (c n)"), in_=wk_view[:, 0]
            )
            nc.sync.dma_start(
                out=wq_sb[:, 0].rearrange("p c n -> p (c n)"), in_=wq_view[:, 0]
            )

            xT = xt_pool.tile([128, CC, N_TOK], mm_dt, tag="xT")
            xt_view = xt_d[:].rearrange("p (t c n) -> p t c n", t=QS, c=CC)
            # token-slice-major so the first k/q matmuls can start early;
            # ts0 in two halves so the first k-chain matmuls overlap the
            # second half's transfer
            nc.sync.dma_start(out=xT[:, 0:4, 0:512], in_=xt_view[:, 0, 0:4])
            nc.sync.dma_start(out=xT[:, 4:8, 0:512], in_=xt_view[:, 0, 4:8])
            nc.sync.dma_start(
                out=wk_sb[:, 1].rearrange("p c n -> p (c n)"), in_=wk_view[:, 1]
            )
            nc.sync.dma_start(
                out=wq_sb[:, 1].rearrange("p c n -> p (c n)"), in_=wq_view[:, 1]
            )
            nc.sync.dma_start(out=wv_sb[:].rearrange("p c n -> p (c n)"), in_=wv_d[:])
            for ts in range(1, QS):
                sl = slice(ts * 512, (ts + 1) * 512)
                nc.sync.dma_start(out=xT[:, :, sl], in_=xt_view[:, ts])
            nc.sync.dma_start(out=wo_sb[:].rearrange("p c n -> p (c n)"), in_=wo_d[:])

            # --- constants / ACT table warm ------------------------------
            ones512 = const_pool.tile([128, 512], mm_dt, tag="ones512")
            nc.vector.memset(ones512[:], 1.0)
            ones256 = ones512[:, 0:256]
            actwarm = const_pool.tile([1, 1], F32, tag="actwarm")
            nc.scalar.activation(
                actwarm[:], ones512[0:1, 0:1], mybir.ActivationFunctionType.Exp
            )
            # dummy matmuls to keep the PE busy through the ~16us input-DMA
            # wall: HAM un-throttles after ~3.4us of sustained activity, so
            # the first real matmuls start at 2.4GHz instead of 1.2.
            for _ in range(3):
                warm_acc = acc_pool.tile([128, 512], F32, tag="acc", name="warm")
                for j in range(8):
                    nc.tensor.matmul(
                        warm_acc[:],
                        ones512[:, 0:128],
                        ones512[:],
                        start=(j == 0),
                        stop=(j == 7),
                    )

            qT = [kq_pool.tile([128, N_TOK], mm_dt, tag=f"qT{p}", name=f"qT{p}") for p in range(2)]
            kT = [kq_pool.tile([128, N_TOK], mm_dt, tag=f"kT{p}", name=f"kT{p}") for p in range(2)]
            oT = [ot_pool.tile([128, N_TOK], mm_dt, tag=f"oT{p}", name=f"oT{p}") for p in range(2)]
            # v_aug: per key-chunk, per head: [64 v columns | 64 ones columns]
            # -> the O^T matmul (M=128) emits softmax denominators on
            # partitions 64..127 of the ots accumulator.
            v_sb = v_pool.tile([128, KC, 4 * 128], mm_dt, tag="v")
            v_aug_view = v_sb[:].rearrange("p k (h e) -> p k h e", h=4)
            for kc in range(KC):
                nc.gpsimd.tensor_copy(
                    v_aug_view[:, kc, :, 64:128],
                    ones256.rearrange("p (h e) -> p h e", h=4),
                )

            # --- emission helpers ---------------------------------------
            def emit_k_slice(p, ts):
                sl = slice(ts * 512, (ts + 1) * 512)
                acc = acc_pool.tile([128, 512], F32, tag="acc", name="acc")
                for cc in range(CC):
                    nc.tensor.matmul(
                        acc[:],
                        wk_sb[:, p, cc, :],
                        xT[:, cc, sl],
                        start=(cc == 0),
                        stop=(cc == CC - 1),
                    )
                nc.vector.tensor_copy(kT[p][:, sl], acc[:])

            def emit_q_slice(p, ts):
                sl = slice(ts * 512, (ts + 1) * 512)
                acc = acc_pool.tile([128, 512], F32, tag="acc", name="acc")
                for cc in range(CC):
                    nc.tensor.matmul(
                        acc[:],
                        wq_sb[:, p, cc, :],
                        xT[:, cc, sl],
                        start=(cc == 0),
                        stop=(cc == CC - 1),
                    )
                nc.vector.tensor_copy(qT[p][:, sl], acc[:])

            def emit_v_kc(kc):
                acc = acc_pool.tile([128, 512], F32, tag="acc", name="acc")
                for cc in range(CC):
                    nc.tensor.matmul(
                        acc[:, 0:256],
                        xT[:, cc, kc * 128:(kc + 1) * 128],
                        wv_sb[:, cc, :],
                        start=(cc == 0),
                        stop=(cc == CC - 1),
                    )
                nc.vector.tensor_copy(
                    v_aug_view[:, kc, :, 0:64],
                    acc[:, 0:256].rearrange("p (h e) -> p h e", h=4),
                )

            def emit_proj_t(t, evict_on_scalar=False, outp=None, dma=True):
                if outp is None:
                    outp = outp_pool.tile([128, C], F16, tag="outp")
                for ns in range(2):
                    po = acc_pool.tile([128, 512], F32, tag="acc", name="po")
                    for p in range(2):
                        nc.tensor.matmul(
                            po[:],
                            oT[p][:, t * 128:(t + 1) * 128],
                            wo_sb[:, p, ns * 512:(ns + 1) * 512],
                            start=(p == 0),
                            stop=(p == 1),
                        )
                    dst = outp[:, ns * 512:(ns + 1) * 512]
                    if evict_on_scalar:
                        nc.scalar.copy(dst, po[:])
                    else:
                        nc.vector.tensor_copy(dst, po[:])
                if dma:
                    nc.sync.dma_start(out=out_d[t * 128:(t + 1) * 128, :], in_=outp[:])

            def emit_proj_pair_fused(t0, alt_evict=True):
                # two token-chunks into one wide tile + a single DMA (the
                # Sync-engine issue cost ~0.6us/DMA dominates small flushes)
                outp2 = outp_pool.tile([128, 2, C], F16, tag="outp2")
                for i, t in enumerate((t0, t0 + 1)):
                    emit_proj_t(t, evict_on_scalar=(alt_evict and i == 1),
                                outp=outp2[:, i], dma=False)
                nc.sync.dma_start(
                    out=out_d[:].rearrange("(c p) n -> p c n", p=128)[
                        :, t0:t0 + 2, :
                    ],
                    in_=outp2[:],
                )

            def unit(ts, p, extras=None, dve_kc=DVE_KC, norm_hooks=None,
                     tail_warm=0):
                """One (q-slice, head-pair) attention unit, fused pipeline.

                norm_hooks: optional pair of callables; normalization is then
                emitted in two 256-token halves with hooks[i] run after half
                i (lets tail proj chunks start before the full normalize).
                """
                extras = extras or {}
                sl = slice(ts * 512, (ts + 1) * 512)
                ots = [
                    ots_pool.tile([128, 512], F32, tag="ots", name=f"ot{h}")
                    for h in range(2)
                ]
                pend = {}

                def emit_o_pair(kc):
                    pt = pend.pop(kc)
                    for h in range(2):
                        a = 2 * p + h
                        nc.tensor.matmul(
                            ots[h][:],
                            v_sb[:, kc, a * 128:(a + 1) * 128],
                            pt[:, h * 512:(h + 1) * 512].bitcast(mm_dt),
                            start=(kc == 0),
                            stop=(kc == KC - 1),
                        )

                # blocked in pairs of kc: S-pairs (64-row mode) grouped, then
                # O-pairs + interleaves (128-row mode) grouped — each
                # 64<->128-row mode switch costs ~100ns of unhidden
                # LDWEIGHTS. Per-kc st tiles keep exp/slot pipelining fine-
                # grained so the PE is not serialized to ACT block pace.
                for b in range(KC // 2):
                    if b >= 2:
                        emit_o_pair(2 * b - 4)
                        emit_o_pair(2 * b - 3)
                    for kc in (2 * b, 2 * b + 1):
                        st = st_pool.tile([128, 1024], F32, tag="st", name="st")
                        for h in range(2):
                            hp = slice(h * 64, (h + 1) * 64)
                            nc.tensor.matmul(
                                st[:, h * 512:(h + 1) * 512],
                                kT[p][hp, kc * 128:(kc + 1) * 128],
                                qT[p][hp, sl],
                                start=True,
                                stop=True,
                            )
                        pt = pt_pool.tile([128, 1024], F16, tag="pt", name="pt")
                        if kc in dve_kc:
                            nc.vector.tensor_scalar(
                                pt[:].bitcast(I16),
                                st[:],
                                A16,
                                B16,
                                mybir.AluOpType.mult,
                                mybir.AluOpType.add,
                            )
                        else:
                            nc.scalar.activation(
                                pt[:],
                                st[:],
                                mybir.ActivationFunctionType.Exp,
                                scale=0.125,
                            )
                        pend[kc] = pt
                    for f in extras.get(b, ()):
                        f()
                for kc in range(KC - 4, KC):
                    emit_o_pair(kc)
                # optional dummy matmuls so the PE stays HAM-warm while the
                # normalize chain runs on DVE (else the tail projs run cold)
                if tail_warm:
                    twarm = acc_pool.tile([128, 512], F32, tag="acc", name="tw")
                    for j in range(tail_warm):
                        nc.tensor.matmul(
                            twarm[:],
                            ones512[:, 0:128],
                            ones512[:],
                            start=(j == 0),
                            stop=(j == tail_warm - 1),
                        )

                # normalize (proven ops only): pack numer/sums across heads
                # with partition-crossing copies, then one reciprocal + mul.
                halves = [(0, 512)] if norm_hooks is None else [(0, 256), (256, 512)]
                for i, (c0, c1) in enumerate(halves):
                    numer = rcp_pool.tile([128, 512], F32, tag="numer")
                    sums = rcp_pool.tile([128, 512], F32, tag="sums")
                    for h in range(2):
                        hq = slice(h * 64, (h + 1) * 64)
                        nc.vector.tensor_copy(numer[hq, c0:c1], ots[h][0:64, c0:c1])
                        nc.vector.tensor_copy(sums[hq, c0:c1], ots[h][64:128, c0:c1])
                    rcp = rcp_pool.tile([128, 512], F32, tag="rcp")
                    nc.vector.reciprocal_approx_fast(
                        out=rcp[:, c0:c1], in_=sums[:, c0:c1]
                    )
                    nc.vector.tensor_mul(
                        oT[p][:, ts * 512 + c0:ts * 512 + c1],
                        numer[:, c0:c1],
                        rcp[:, c0:c1],
                    )
                    if norm_hooks is not None:
                        norm_hooks[i]()

            # --- master schedule ----------------------------------------
            emit_k_slice(0, 0)
            emit_q_slice(0, 0)

            # extras are keyed by block index b (= kc pair 2b, 2b+1)
            # A = (ts0, p0): absorbs all v chunks + rest of k(p0)/q(p0,1)
            unit(0, 0, {
                0: [lambda: emit_v_kc(0), lambda: emit_k_slice(0, 1),
                    lambda: emit_v_kc(1)],
                1: [lambda: emit_v_kc(2), lambda: emit_k_slice(0, 2),
                    lambda: emit_v_kc(3)],
                2: [lambda: emit_v_kc(4), lambda: emit_k_slice(0, 3),
                    lambda: emit_v_kc(5)],
                3: [lambda: emit_q_slice(0, 1), lambda: emit_v_kc(6),
                    lambda: emit_v_kc(7)],
                4: [lambda: emit_v_kc(8), lambda: emit_v_kc(9)],
                5: [lambda: emit_v_kc(10), lambda: emit_v_kc(11)],
                6: [lambda: emit_v_kc(12), lambda: emit_v_kc(13)],
                7: [lambda: emit_v_kc(14), lambda: emit_v_kc(15)],
            })
            # B = (ts1, p0): absorbs k(p1) + q(p1,0) + q(p0,2)
            unit(1, 0, {
                0: [lambda: emit_k_slice(1, 0)],
                1: [lambda: emit_k_slice(1, 1)],
                2: [lambda: emit_k_slice(1, 2)],
                3: [lambda: emit_k_slice(1, 3)],
                4: [lambda: emit_q_slice(1, 0)],
                5: [lambda: emit_q_slice(0, 2)],
            })
            # C = (ts0, p1)
            unit(0, 1, {
                1: [lambda: emit_q_slice(0, 3)],
                4: [lambda: emit_q_slice(1, 1)],
            }, dve_kc=(3, 6, 9, 12, 14))
            # D = (ts2, p0): proj(0) interleaved
            unit(2, 0, {
                0: [lambda: emit_proj_t(0)],
                2: [lambda: emit_proj_t(1)],
                4: [lambda: emit_proj_t(2)],
                5: [lambda: emit_proj_t(3)],
                6: [lambda: emit_q_slice(1, 2)],
            }, dve_kc=(3, 6, 9, 12, 14))
            # E = (ts1, p1)
            unit(1, 1, {
                1: [lambda: emit_q_slice(1, 3)],
            }, dve_kc=(3, 6, 9, 12, 14))
            # F = (ts3, p0): proj(1) interleaved
            unit(3, 0, {
                0: [lambda: emit_proj_t(4)],
                2: [lambda: emit_proj_t(5)],
                4: [lambda: emit_proj_t(6)],
                5: [lambda: emit_proj_t(7)],
            })
            # G = (ts2, p1): mid-unit DVE offload (DVE FIFO is free mid-unit)
            unit(2, 1, {}, dve_kc=(1, 3, 5, 7, 9, 12, 14))
            # H = (ts3, p1): proj(2) interleaved, evicts split across ACT/DVE
            unit(3, 1, {
                0: [lambda: emit_proj_t(8)],
                2: [lambda: emit_proj_t(9, evict_on_scalar=True)],
                4: [lambda: emit_proj_t(10)],
                5: [lambda: emit_proj_t(11, evict_on_scalar=True)],
            }, dve_kc=(1, 3, 5, 7, 9, 11, 13, 15), norm_hooks=(
                # tail proj(3): two fused token-chunks per normalize half,
                # evicts alternating DVE/ACT, one DMA per pair.
                lambda: emit_proj_pair_fused(12),
                lambda: emit_proj_pair_fused(14),
            ), tail_warm=14)

    nc.compile()
    return nc


def _shard_inputs(x, w_qkv, w_out):
    # [2048, 1024] -> [128, QS*CC*512]: partition p holds, per token-slice,
    # per contraction chunk, 512 contiguous tokens (8KB lines per DMA).
    xts = [
        np.ascontiguousarray(
            x[b].reshape(QS, 512, CC, 128).transpose(3, 0, 2, 1).reshape(128, -1)
        ).astype(np.float16)
        for b in range(2)
    ]
    in_maps = []
    for i in range(8):
        b, g = divmod(i, 4)
        cs = slice(256 * g, 256 * (g + 1))

        def shuf(w):
            # [1024, n] -> [128, CC * n] with chunk-major columns
            n = w.shape[1]
            return np.ascontiguousarray(
                w.reshape(CC, 128, n).transpose(1, 0, 2).reshape(128, CC * n)
            ).astype(np.float16)

        def shuf_ph(w):
            # [1024, 256] -> [128, 2*CC*128], p-half-major then chunk-major
            return np.ascontiguousarray(
                w.reshape(CC, 128, 2, 128).transpose(1, 2, 0, 3).reshape(128, -1)
            ).astype(np.float16)

        in_maps.append({
            "xt": xts[b],
            "wq": shuf_ph(w_qkv[:, cs]),
            "wk": shuf_ph(w_qkv[:, 1024 + 256 * g:1024 + 256 * (g + 1)]),
            "wv": shuf(w_qkv[:, 2048 + 256 * g:2048 + 256 * (g + 1)]),
            "wo": np.ascontiguousarray(
                w_out[cs, :].reshape(2, 128, 1024).transpose(1, 0, 2).reshape(128, 2048)
            ).astype(np.float16),
        })
    return in_maps


def kernel(x, w_qkv, w_out, b_out):
    global _COMPILED
    x = np.asarray(x, np.float32)
    w_qkv = np.asarray(w_qkv, np.float32)
    w_out = np.asarray(w_out, np.float32)
    b_out = np.asarray(b_out, np.float32)

    if _COMPILED is None:
        _COMPILED = build_nc()
    nc = _COMPILED

    in_maps = _shard_inputs(x, w_qkv, w_out)
    res = run_bass_kernel_spmd(nc, in_maps, core_ids=list(range(8)))
    out = np.zeros((2, N_TOK, C), np.float32)
    for i in range(8):
        b = i // 4
        out[b] += res.results[i]["out"].astype(np.float32)
    out += b_out[None, None, :]
    return out
